# revision 6
# baseline (speedup 1.0000x reference)
"""Sliding-window attention Trainium2 Bass kernel (v3).

Problem: B=4, H=32, L=4096, D=128, window=512.
reference: attends over the LAST w=512 key/value positions; query row i may
only see window slot j when j <= i (slots are key positions L-w+j).

Sharding: B*H = 128 (b,h) pairs split across 8 cores -> 16 heads/core.
Pure data parallelism, no collectives.

Per-group (512 queries) pipeline (Q/K/V/P in fp16, accum f32):
  S^T chunks [128, 512] = (K^T chunk)^T . (Q^T group)   4 MM   (PE -> PSUM)
  P^T = exp(S^T / sqrt(D)) -> fp16                 2 ACT ops (the wall)
  g==0: causal mask applied to P^T post-exp: zero rectangles (Pool memset)
        + triangle multiply on the diagonal blocks (DVE) — ACT never waits
  a1 = p0+p1 (Pool), a2 = p2+p3 (DVE)              partial chunk sums
  R[128,512] = ones128^T @ a1 + ones128^T @ a2     2 MM: rowsum+broadcast
  rc = recip_approx_fast(R)                              (DVE)
  O^T [128, 512] += V_c^T @ P_c^T                  4 MM   (PE -> PSUM)
  out = O^T * rc                                         (DVE, PSUM->SBUF)

a2 is emitted after back(prev) so recip/mul(prev) aren't head-of-line
blocked on DVE behind a2's wait for exp1. Head-0 load is split (kt+q0 first)
so the first S matmul doesn't wait the whole 13-DMA batch, and a dummy exp
at program start prefires the 1.3us ACT table load.
PSUM: S 2x[128,1024] ring2 (4 banks) + O [128,512] ring3 + R ring1 = 8 banks.
"""

import math
from contextlib import ExitStack

import numpy as np

N_CORES = 8
B, H, L, D = 4, 32, 4096, 128
W = 512            # window
HEADS_PER_CORE = (B * H) // N_CORES   # 16
QG = 512           # queries per group
NG = L // QG       # groups per head (8)
NCHUNK = W // 128  # 4 window chunks
SCALE = 1.0 / math.sqrt(D)

_COMPILED = None


def _build():
    import concourse.tile as tile
    from concourse import bacc, mybir

    nc = bacc.Bacc("TRN2", target_bir_lowering=False, debug=False,
                   num_devices=N_CORES)

    f16 = mybir.dt.float16
    f32 = mybir.dt.float32

    qT = nc.dram_tensor("qT", [HEADS_PER_CORE, D, L], f16, kind="ExternalInput").ap()
    kT = nc.dram_tensor("kT", [HEADS_PER_CORE, D, W], f16, kind="ExternalInput").ap()
    v = nc.dram_tensor("v", [HEADS_PER_CORE, W, D], f16, kind="ExternalInput").ap()
    tri = nc.dram_tensor("tri", [128, 128], f16, kind="ExternalInput").ap()
    ones = nc.dram_tensor("ones", [128, 128], f16, kind="ExternalInput").ap()
    outT = nc.dram_tensor("outT", [HEADS_PER_CORE, D, L], f32, kind="ExternalOutput").ap()

    with tile.TileContext(nc) as tc:
        with ExitStack() as ctx:
            const = ctx.enter_context(tc.tile_pool(name="const", bufs=1))
            kt_pool = ctx.enter_context(tc.tile_pool(name="kt", bufs=2))
            v_pool = ctx.enter_context(tc.tile_pool(name="v", bufs=2))
            q_pool = ctx.enter_context(tc.tile_pool(name="q", bufs=2 * NG))
            o_pool = ctx.enter_context(tc.tile_pool(name="o", bufs=3))
            p_pool = ctx.enter_context(tc.tile_pool(name="p", bufs=4))
            a_pool = ctx.enter_context(tc.tile_pool(name="a", bufs=3))
            rc_pool = ctx.enter_context(tc.tile_pool(name="rc", bufs=3))
            s_psum = ctx.enter_context(tc.tile_pool(name="s_ps", bufs=2, space="PSUM"))
            o_psum = ctx.enter_context(tc.tile_pool(name="o_ps", bufs=3, space="PSUM"))
            r_psum = ctx.enter_context(tc.tile_pool(name="r_ps", bufs=1, space="PSUM"))

            # prefire the ACT exp table load on a scratch tile
            warm_t = const.tile([1, 2], f32, tag="warm")
            nc.gpsimd.memset(warm_t[:], 0)
            nc.scalar.activation(warm_t[:, 0:1], warm_t[:, 1:2],
                                 mybir.ActivationFunctionType.Exp)

            tri_t = const.tile([128, 128], f16, tag="tri")
            nc.gpsimd.dma_start(tri_t[:], tri[:])
            ones_t = const.tile([128, 128], f16, tag="ones")
            nc.gpsimd.dma_start(ones_t[:], ones[:])

            head_tiles = {}

            def load_kq0(h):
                """kt + first q tile — just enough for front(h, 0)."""
                kt_t = kt_pool.tile([128, W], f16, tag="kt")
                nc.sync.dma_start(kt_t[:], kT[h])
                qt0 = q_pool.tile([128, QG], f16, tag="q")
                nc.sync.dma_start(qt0[:], qT[h, :, 0:QG])
                head_tiles[h] = [kt_t, None, [qt0]]

            def load_v(h, c0, c1):
                ht = head_tiles[h]
                if ht[1] is None:
                    ht[1] = v_pool.tile([128, NCHUNK * D], f16, tag="v", name="v_t")
                for c in range(c0, c1):
                    nc.sync.dma_start(ht[1][:, c * D:(c + 1) * D],
                                      v[h, c * 128:(c + 1) * 128, :])

            def load_q(h, i0, i1):
                ht = head_tiles[h]
                for i in range(i0, min(i1, NG)):
                    qt_t = q_pool.tile([128, QG], f16, tag="q", name="qt_t")
                    nc.sync.dma_start(qt_t[:], qT[h, :, i * QG:(i + 1) * QG])
                    ht[2].append(qt_t)

            def emit_loads(h, g):
                """<=2 load DMAs per iteration for head h+1 (plus the head-0
                bootstrap) so the sync DMA queue never backs up and stalls
                S matmuls via coalesced completion semaphores."""
                if h == 0:
                    # bootstrap: finish head 0's own tiles first
                    if g == 0:
                        load_v(0, 0, 4)
                        load_q(0, 1, 2)
                    elif g == 1:
                        load_q(0, 2, 4)
                    elif g == 2:
                        load_q(0, 4, 6)
                    elif g == 3:
                        load_q(0, 6, 8)
                nxt = h + 1
                if nxt >= HEADS_PER_CORE:
                    return
                if g == 1:
                    load_kq0(nxt)
                elif g == 2:
                    load_v(nxt, 0, 2)
                elif g == 3:
                    load_v(nxt, 2, 4)
                elif g == 4:
                    load_q(nxt, 1, 3)
                elif g == 5:
                    load_q(nxt, 3, 5)
                elif g == 6:
                    load_q(nxt, 5, 7)
                elif g == 7:
                    load_q(nxt, 7, 8)

            def emit_front(h, g):
                """S matmuls + exp + post-exp mask (g==0) + a1 partial sum."""
                kt_t, v_t, qt_tiles = head_tiles[h]
                qt_t = qt_tiles[g]
                halves = []
                for half in range(2):
                    s_ps = s_psum.tile([128, 2 * QG], f32, tag="s")
                    for ci in range(2):
                        c = half * 2 + ci
                        nc.tensor.matmul(
                            s_ps[:, ci * QG:(ci + 1) * QG],
                            lhsT=kt_t[:, c * 128:(c + 1) * 128],
                            rhs=qt_t[:],
                            start=True, stop=True,
                        )
                    halves.append(s_ps)
                p_t = p_pool.tile([128, NCHUNK * QG], f16, tag="p")
                a1_t = a_pool.tile([128, QG], f16, tag="a1")
                a2_t = a_pool.tile([128, QG], f16, tag="a2")
                for half in range(2):
                    nc.scalar.activation(
                        p_t[:, half * 2 * QG:(half + 1) * 2 * QG],
                        halves[half][:],
                        mybir.ActivationFunctionType.Exp, scale=SCALE)
                    if g == 0:
                        for ci in range(2):
                            c = half * 2 + ci
                            base = c * QG
                            if c > 0:
                                # queries [0, c*128) see nothing of chunk c
                                nc.gpsimd.memset(p_t[:, base:base + c * 128], 0)
                            # diagonal block: query c*128+qq sees slot c*128+jj
                            # iff qq >= jj
                            dblk = slice(base + c * 128, base + (c + 1) * 128)
                            nc.vector.tensor_mul(p_t[:, dblk], p_t[:, dblk],
                                                 tri_t[:])
                    if half == 0:
                        # partial sum of chunks 0+1 on the (slack) Pool engine
                        nc.gpsimd.tensor_add(a1_t[:], p_t[:, 0:QG],
                                             p_t[:, QG:2 * QG])
                return (h, g, p_t, a1_t, a2_t)

            def emit_sums(stage):
                """a2 partial sum (DVE) — emitted after back(prev) so it
                doesn't block recip/mul(prev) in the DVE queue."""
                h, g, p_t, a1_t, a2_t = stage
                nc.vector.tensor_add(a2_t[:], p_t[:, 2 * QG:3 * QG],
                                     p_t[:, 3 * QG:4 * QG])

            def emit_back(stage):
                """PV + rowsum-broadcast + recip + normalize for a front stage."""
                h, g, p_t, a1_t, a2_t = stage
                kt_t, v_t, qt_tiles = head_tiles[h]
                o_ps = o_psum.tile([128, QG], f32, tag="ops")
                for c in range(NCHUNK):
                    nc.tensor.matmul(
                        o_ps[:],
                        lhsT=v_t[:, c * D:(c + 1) * D],
                        rhs=p_t[:, c * QG:(c + 1) * QG],
                        start=(c == 0), stop=(c == NCHUNK - 1),
                    )
                r_ps = r_psum.tile([128, QG], f32, tag="r")
                nc.tensor.matmul(r_ps[:], lhsT=ones_t[:], rhs=a1_t[:],
                                 start=True, stop=False)
                nc.tensor.matmul(r_ps[:], lhsT=ones_t[:], rhs=a2_t[:],
                                 start=False, stop=True)
                rc_t = rc_pool.tile([128, QG], f32, tag="rc")
                nc.vector.reciprocal_approx_fast(rc_t[:], r_ps[:])
                o_t = o_pool.tile([128, QG], f32, tag="o")
                nc.vector.tensor_mul(o_t[:], o_ps[:], rc_t[:])
                nc.sync.dma_start(outT[h, :, g * QG:(g + 1) * QG], o_t[:])
                if g == NG - 1:
                    del head_tiles[h]

            prev = None
            load_kq0(0)
            for it in range(HEADS_PER_CORE * NG):
                h, g = divmod(it, NG)
                cur = emit_front(h, g)
                if prev is not None:
                    emit_back(prev)
                emit_sums(cur)
                emit_loads(h, g)
                prev = cur
            emit_back(prev)

    nc.compile()
    return nc


def _get_compiled():
    global _COMPILED
    if _COMPILED is None:
        _COMPILED = _build()
    return _COMPILED


def _make_in_maps(query, keys, values):
    q = np.asarray(query, dtype=np.float32)
    k = np.asarray(keys, dtype=np.float32)
    v = np.asarray(values, dtype=np.float32)

    qf = q.reshape(B * H, L, D)
    kf = k.reshape(B * H, L, D)[:, L - W:, :]
    vf = v.reshape(B * H, L, D)[:, L - W:, :]

    # within a diagonal 128x128 block: query qq sees slot jj iff qq >= jj
    tri = (np.arange(128)[None, :] >= np.arange(128)[:, None]).astype(np.float16)
    ones = np.ones((128, 128), dtype=np.float16)

    in_maps = []
    for core in range(N_CORES):
        s = slice(core * HEADS_PER_CORE, (core + 1) * HEADS_PER_CORE)
        in_maps.append({
            "qT": np.ascontiguousarray(qf[s].transpose(0, 2, 1)).astype(np.float16),
            "kT": np.ascontiguousarray(kf[s].transpose(0, 2, 1)).astype(np.float16),
            "v": np.ascontiguousarray(vf[s]).astype(np.float16),
            "tri": tri,
            "ones": ones,
        })
    return in_maps


def kernel(query, keys, values, window_size):
    from concourse.bass_utils import run_bass_kernel_spmd

    w = int(window_size)
    assert np.asarray(query).shape == (B, H, L, D) and w == W

    nc = _get_compiled()
    in_maps = _make_in_maps(query, keys, values)
    res = run_bass_kernel_spmd(nc, in_maps, core_ids=list(range(N_CORES)))
    outs = [res.results[c]["outT"].transpose(0, 2, 1) for c in range(N_CORES)]
    return np.concatenate(outs, axis=0).reshape(B, H, L, D)


# revision 7
# speedup vs baseline: 1.0353x; 1.0353x over previous
"""Sliding-window attention Trainium2 Bass kernel (v3).

Problem: B=4, H=32, L=4096, D=128, window=512.
reference: attends over the LAST w=512 key/value positions; query row i may
only see window slot j when j <= i (slots are key positions L-w+j).

Sharding: B*H = 128 (b,h) pairs split across 8 cores -> 16 heads/core.
Pure data parallelism, no collectives.

Per-group (512 queries) pipeline (Q/K/V/P in fp16, accum f32):
  S^T chunks [128, 512] = (K^T chunk)^T . (Q^T group)   4 MM   (PE -> PSUM)
  P^T = exp(S^T / sqrt(D)) -> fp16                 2 ACT ops (the wall)
  g==0: causal mask applied to P^T post-exp: zero rectangles (Pool memset)
        + triangle multiply on the diagonal blocks (DVE) — ACT never waits
  a1 = p0+p1 (Pool), a2 = p2+p3 (DVE)              partial chunk sums
  R[128,512] = ones128^T @ a1 + ones128^T @ a2     2 MM: rowsum+broadcast
  rc = recip_approx_fast(R)                              (DVE)
  O^T [128, 512] += V_c^T @ P_c^T                  4 MM   (PE -> PSUM)
  out = O^T * rc                                         (DVE, PSUM->SBUF)

a2 is emitted after back(prev) so recip/mul(prev) aren't head-of-line
blocked on DVE behind a2's wait for exp1. Head-0 load is split (kt+q0 first)
so the first S matmul doesn't wait the whole 13-DMA batch, and a dummy exp
at program start prefires the 1.3us ACT table load.
PSUM: S 2x[128,1024] ring2 (4 banks) + O [128,512] ring3 + R ring1 = 8 banks.
"""

import math
from contextlib import ExitStack

import numpy as np

N_CORES = 8
B, H, L, D = 4, 32, 4096, 128
W = 512            # window
HEADS_PER_CORE = (B * H) // N_CORES   # 16
QG = 512           # queries per group
NG = L // QG       # groups per head (8)
NCHUNK = W // 128  # 4 window chunks
SCALE = 1.0 / math.sqrt(D)

_COMPILED = None


def _build():
    import concourse.tile as tile
    from concourse import bacc, mybir

    nc = bacc.Bacc("TRN2", target_bir_lowering=False, debug=False,
                   num_devices=N_CORES)

    f16 = mybir.dt.float16
    f32 = mybir.dt.float32

    qT = nc.dram_tensor("qT", [HEADS_PER_CORE, D, L], f16, kind="ExternalInput").ap()
    kT = nc.dram_tensor("kT", [HEADS_PER_CORE, D, W], f16, kind="ExternalInput").ap()
    v = nc.dram_tensor("v", [HEADS_PER_CORE, W, D], f16, kind="ExternalInput").ap()
    tri = nc.dram_tensor("tri", [128, 128], f16, kind="ExternalInput").ap()
    ones = nc.dram_tensor("ones", [128, 128], f16, kind="ExternalInput").ap()
    outT = nc.dram_tensor("outT", [HEADS_PER_CORE, D, L], f32, kind="ExternalOutput").ap()

    with tile.TileContext(nc) as tc:
        with ExitStack() as ctx:
            const = ctx.enter_context(tc.tile_pool(name="const", bufs=1))
            kt_pool = ctx.enter_context(tc.tile_pool(name="kt", bufs=2))
            v_pool = ctx.enter_context(tc.tile_pool(name="v", bufs=2))
            q_pool = ctx.enter_context(tc.tile_pool(name="q", bufs=2 * NG))
            o_pool = ctx.enter_context(tc.tile_pool(name="o", bufs=3))
            p_pool = ctx.enter_context(tc.tile_pool(name="p", bufs=4))
            a_pool = ctx.enter_context(tc.tile_pool(name="a", bufs=3))
            rc_pool = ctx.enter_context(tc.tile_pool(name="rc", bufs=3))
            s_psum = ctx.enter_context(tc.tile_pool(name="s_ps", bufs=2, space="PSUM"))
            o_psum = ctx.enter_context(tc.tile_pool(name="o_ps", bufs=3, space="PSUM"))
            r_psum = ctx.enter_context(tc.tile_pool(name="r_ps", bufs=1, space="PSUM"))

            # prefire the ACT exp table load on a scratch tile
            warm_t = const.tile([1, 2], f32, tag="warm")
            nc.gpsimd.memset(warm_t[:], 0)
            nc.scalar.activation(warm_t[:, 0:1], warm_t[:, 1:2],
                                 mybir.ActivationFunctionType.Exp)

            tri_t = const.tile([128, 128], f16, tag="tri")
            nc.gpsimd.dma_start(tri_t[:], tri[:])
            ones_t = const.tile([128, 128], f16, tag="ones")
            nc.gpsimd.dma_start(ones_t[:], ones[:])

            head_tiles = {}

            def load_kq0(h):
                """kt + first q tile — just enough for front(h, 0)."""
                kt_t = kt_pool.tile([128, W], f16, tag="kt")
                nc.sync.dma_start(kt_t[:], kT[h])
                qt0 = q_pool.tile([128, QG], f16, tag="q")
                nc.sync.dma_start(qt0[:], qT[h, :, 0:QG])
                head_tiles[h] = [kt_t, None, [qt0]]

            def load_v(h, c0, c1):
                ht = head_tiles[h]
                if ht[1] is None:
                    ht[1] = v_pool.tile([128, NCHUNK * D], f16, tag="v", name="v_t")
                for c in range(c0, c1):
                    nc.sync.dma_start(ht[1][:, c * D:(c + 1) * D],
                                      v[h, c * 128:(c + 1) * 128, :])

            def load_q(h, i0, i1):
                ht = head_tiles[h]
                for i in range(i0, min(i1, NG)):
                    qt_t = q_pool.tile([128, QG], f16, tag="q", name="qt_t")
                    nc.sync.dma_start(qt_t[:], qT[h, :, i * QG:(i + 1) * QG])
                    ht[2].append(qt_t)

            def emit_loads(h, g):
                """<=2 load DMAs per iteration for head h+1 (plus the head-0
                bootstrap) so the sync DMA queue never backs up and stalls
                S matmuls via coalesced completion semaphores."""
                if h == 0:
                    # bootstrap: finish head 0's own tiles first
                    if g == 0:
                        load_v(0, 0, 4)
                        load_q(0, 1, 2)
                    elif g == 1:
                        load_q(0, 2, 4)
                    elif g == 2:
                        load_q(0, 4, 6)
                    elif g == 3:
                        load_q(0, 6, 8)
                nxt = h + 1
                if nxt >= HEADS_PER_CORE:
                    return
                if g == 1:
                    load_kq0(nxt)
                elif g == 2:
                    load_v(nxt, 0, 2)
                elif g == 3:
                    load_v(nxt, 2, 4)
                elif g == 4:
                    load_q(nxt, 1, 3)
                elif g == 5:
                    load_q(nxt, 3, 5)
                elif g == 6:
                    load_q(nxt, 5, 7)
                elif g == 7:
                    load_q(nxt, 7, 8)

            def emit_front(h, g):
                """S matmuls + exp + post-exp mask (g==0) + a1 partial sum."""
                kt_t, v_t, qt_tiles = head_tiles[h]
                qt_t = qt_tiles[g]
                halves = []
                for half in range(2):
                    s_ps = s_psum.tile([128, 2 * QG], f32, tag="s")
                    for ci in range(2):
                        c = half * 2 + ci
                        nc.tensor.matmul(
                            s_ps[:, ci * QG:(ci + 1) * QG],
                            lhsT=kt_t[:, c * 128:(c + 1) * 128],
                            rhs=qt_t[:],
                            start=True, stop=True,
                        )
                    halves.append(s_ps)
                p_t = p_pool.tile([128, NCHUNK * QG], f16, tag="p")
                a1_t = a_pool.tile([128, QG], f16, tag="a1")
                a2_t = a_pool.tile([128, QG], f16, tag="a2")
                for half in range(2):
                    nc.scalar.activation(
                        p_t[:, half * 2 * QG:(half + 1) * 2 * QG],
                        halves[half][:],
                        mybir.ActivationFunctionType.Exp, scale=SCALE)
                    if g == 0:
                        for ci in range(2):
                            c = half * 2 + ci
                            base = c * QG
                            if c > 0:
                                # queries [0, c*128) see nothing of chunk c
                                nc.gpsimd.memset(p_t[:, base:base + c * 128], 0)
                            # diagonal block: query c*128+qq sees slot c*128+jj
                            # iff qq >= jj
                            dblk = slice(base + c * 128, base + (c + 1) * 128)
                            nc.vector.tensor_mul(p_t[:, dblk], p_t[:, dblk],
                                                 tri_t[:])
                    if half == 0:
                        # partial sum of chunks 0+1 on the (slack) Pool engine
                        nc.gpsimd.tensor_add(a1_t[:], p_t[:, 0:QG],
                                             p_t[:, QG:2 * QG])
                return (h, g, p_t, a1_t, a2_t)

            def emit_sums(stage):
                """a2 partial sum (DVE) — emitted after back(prev) so it
                doesn't block recip/mul(prev) in the DVE queue."""
                h, g, p_t, a1_t, a2_t = stage
                nc.vector.tensor_add(a2_t[:], p_t[:, 2 * QG:3 * QG],
                                     p_t[:, 3 * QG:4 * QG])

            def emit_back_pv(stage):
                """PV accumulation for a front stage (one group behind)."""
                h, g, p_t, a1_t, a2_t = stage
                v_t = head_tiles[h][1]
                o_ps = o_psum.tile([128, QG], f32, tag="ops")
                for c in range(NCHUNK):
                    nc.tensor.matmul(
                        o_ps[:],
                        lhsT=v_t[:, c * D:(c + 1) * D],
                        rhs=p_t[:, c * QG:(c + 1) * QG],
                        start=(c == 0), stop=(c == NCHUNK - 1),
                    )
                return o_ps

            def emit_back_rs(stage, o_ps):
                """Rowsum-broadcast + recip + normalize (two groups behind, so
                the RS matmuls never stall the PE queue waiting on a2)."""
                h, g, p_t, a1_t, a2_t = stage
                r_ps = r_psum.tile([128, QG], f32, tag="r")
                nc.tensor.matmul(r_ps[:], lhsT=ones_t[:], rhs=a1_t[:],
                                 start=True, stop=False)
                nc.tensor.matmul(r_ps[:], lhsT=ones_t[:], rhs=a2_t[:],
                                 start=False, stop=True)
                rc_t = rc_pool.tile([128, QG], f32, tag="rc")
                nc.vector.reciprocal_approx_fast(rc_t[:], r_ps[:])
                o_t = o_pool.tile([128, QG], f32, tag="o")
                nc.vector.tensor_mul(o_t[:], o_ps[:], rc_t[:])
                nc.sync.dma_start(outT[h, :, g * QG:(g + 1) * QG], o_t[:])
                if g == NG - 1:
                    del head_tiles[h]

            prev = prev2 = None
            o_prev = o_prev2 = None
            load_kq0(0)
            for it in range(HEADS_PER_CORE * NG):
                h, g = divmod(it, NG)
                cur = emit_front(h, g)
                if prev is not None:
                    o_prev = emit_back_pv(prev)
                if prev2 is not None:
                    emit_back_rs(prev2, o_prev2)
                emit_sums(cur)
                emit_loads(h, g)
                prev2, prev = prev, cur
                o_prev2 = o_prev
            o_prev = emit_back_pv(prev)
            emit_back_rs(prev2, o_prev2)
            emit_back_rs(prev, o_prev)

    nc.compile()
    return nc


def _get_compiled():
    global _COMPILED
    if _COMPILED is None:
        _COMPILED = _build()
    return _COMPILED


def _make_in_maps(query, keys, values):
    q = np.asarray(query, dtype=np.float32)
    k = np.asarray(keys, dtype=np.float32)
    v = np.asarray(values, dtype=np.float32)

    qf = q.reshape(B * H, L, D)
    kf = k.reshape(B * H, L, D)[:, L - W:, :]
    vf = v.reshape(B * H, L, D)[:, L - W:, :]

    # within a diagonal 128x128 block: query qq sees slot jj iff qq >= jj
    tri = (np.arange(128)[None, :] >= np.arange(128)[:, None]).astype(np.float16)
    ones = np.ones((128, 128), dtype=np.float16)

    in_maps = []
    for core in range(N_CORES):
        s = slice(core * HEADS_PER_CORE, (core + 1) * HEADS_PER_CORE)
        in_maps.append({
            "qT": np.ascontiguousarray(qf[s].transpose(0, 2, 1)).astype(np.float16),
            "kT": np.ascontiguousarray(kf[s].transpose(0, 2, 1)).astype(np.float16),
            "v": np.ascontiguousarray(vf[s]).astype(np.float16),
            "tri": tri,
            "ones": ones,
        })
    return in_maps


def kernel(query, keys, values, window_size):
    from concourse.bass_utils import run_bass_kernel_spmd

    w = int(window_size)
    assert np.asarray(query).shape == (B, H, L, D) and w == W

    nc = _get_compiled()
    in_maps = _make_in_maps(query, keys, values)
    res = run_bass_kernel_spmd(nc, in_maps, core_ids=list(range(N_CORES)))
    outs = [res.results[c]["outT"].transpose(0, 2, 1) for c in range(N_CORES)]
    return np.concatenate(outs, axis=0).reshape(B, H, L, D)
